# revision 55
# baseline (speedup 1.0000x reference)
"""Multi-head attention, tensor-parallel over heads on 8 Trainium2 NeuronCores.

Contract: kernel(**inputs) takes the FULL unsharded inputs from
reference.setup_inputs() and returns the FULL [2, 2048, 1024] fp32 output.

Sharding: 16 heads / 8 cores = 2 heads per core (tensor parallel).
Each core receives the full (host-transposed, bf16-cast) activations and its
2-head slice of wq/wk/wv plus the matching wo columns; it computes
  qhT/khT = (x @ Wq_c^T)^T      (head-dim on partitions)
  vh      =  x @ Wv_c^T          (seq on partitions, direct orientation)
  S^T     = khT^T.T @ qhT        (k-seq on PSUM partitions, 1 kt per group,
                                  both heads sharing one psum tile so bufs=2
                                  truly double-buffers scores against exp)
  es      = exp(S^T / 8)         (no max subtraction: logits ~ N(0,1))
  att^T   = [vh | 1]^T-stationary AV matmul accumulated over kt (row 64 =
             softmax denominator; vh must be the STATIONARY operand: the
             lowering's Ldweights split does not carry semaphore waits, so a
             freshly-written stationary like es races on the in-order PE)
  att     = att^T[:64] * gpsimd-broadcast(recip(att^T[64]))
  partial = att^T @ wo_c^T       (stored bf16)
The host sums the 8 partials and adds the (linear) bv/bo bias terms.

Emission is software-pipelined (PIPE=4): scores/exp of group i are emitted
four groups before their AV matmuls so the in-order PE queue always has
independent work ahead of any exp-gated instruction; the out-projection of a
q-group trails its normalize by OP_DELAY flushes. Background projection work
and x prefetch for the next batch are interleaved at group boundaries on the
SP DMA queue (dependent DMAs never head-of-line block the prefetch), and the
exp zero-bias/weights ride in packed input tensors so no const-AP DMA stalls
the first exp.
"""

import sys
from collections import defaultdict

import numpy as np

sys.path.insert(0, "/opt/trn_rl_repo")

import ml_dtypes  # noqa: E402

import concourse.bacc as bacc  # noqa: E402
import concourse.mybir as mybir  # noqa: E402
import concourse.tile as tile  # noqa: E402
from concourse.bass_utils import run_bass_kernel_spmd  # noqa: E402

D_MODEL = 1024
NUM_HEADS = 16
DEPTH = 64
B, S = 2, 2048
BS = B * S  # 4096
NCORES = 8
HPC = NUM_HEADS // NCORES  # 2 heads per core
HD = HPC * DEPTH  # 128 head dims per core
KC = D_MODEL // 128  # 8 contraction chunks of 128
NT = 512  # x-chunk / moving-free tile
QT = S // NT  # 4 q-groups per batch
KT = S // 128  # 16 k-tiles per batch
KT2 = KT // 2  # 8 kt-pair groups per q-group
P = 128

FP32 = mybir.dt.float32
BF16 = mybir.dt.bfloat16
NPBF = ml_dtypes.bfloat16


def _build_program(loop_iters=1):
    nc = bacc.Bacc(
        "TRN2", target_bir_lowering=False, debug=False, num_devices=NCORES
    )
    io = {}
    io["xqT"] = nc.dram_tensor("xqT", [D_MODEL, BS], BF16, kind="ExternalInput").ap()
    io["xkT"] = nc.dram_tensor("xkT", [D_MODEL, BS], BF16, kind="ExternalInput").ap()
    io["xvT"] = nc.dram_tensor("xvT", [D_MODEL, BS], BF16, kind="ExternalInput").ap()
    # qkv weights packed host-side as [p, kc, 3, hd] (one 6KB-row DMA);
    # bias vectors packed as [p, 3] = (zb, bk, bq)
    io["wkp"] = nc.dram_tensor("wkp", [P, KC * HD], BF16, kind="ExternalInput").ap()
    io["wqv"] = nc.dram_tensor(
        "wqv", [P, KC * 2 * HD], BF16, kind="ExternalInput"
    ).ap()
    io["woT"] = nc.dram_tensor("woT", [HD, D_MODEL], BF16, kind="ExternalInput").ap()
    io["bias3"] = nc.dram_tensor("bias3", [HD, 3], FP32, kind="ExternalInput").ap()
    io["out"] = nc.dram_tensor("out", [BS, D_MODEL], BF16, kind="ExternalOutput").ap()

    with tile.TileContext(nc, trace_sim=False) as tc:
        if loop_iters > 1:
            with tc.For_i(0, loop_iters, 1):
                _emit(tc, nc, io)
        else:
            _emit(tc, nc, io)
    nc.compile()
    return nc


def _emit(tc, nc, io):
    EXP = mybir.ActivationFunctionType.Exp
    with (
        tc.tile_pool(name="const", bufs=1) as cpool,
        tc.tile_pool(name="acts", bufs=1) as apool,
        tc.tile_pool(name="xin", bufs=10) as xpool,
        tc.tile_pool(name="es", bufs=7) as spool,
        tc.tile_pool(name="sm", bufs=4) as smpool,
        tc.tile_pool(name="ot", bufs=2) as opool,
        tc.tile_pool(name="ps", bufs=2, space="PSUM") as pspool,
        tc.tile_pool(name="pv", bufs=2, space="PSUM") as pavpool,
        tc.tile_pool(name="po", bufs=2, space="PSUM") as popool,
    ):
        # --- constants (emission order = DMA service order on the shared bus;
        #     bias+qkv weights + first x chunks first, wo late). The zero exp
        #     bias rides in bias3 col 0: a float bias would lower to a
        #     const-AP DMA queued behind all startup x DMAs.
        wk_sb = cpool.tile([P, KC, HD], BF16, tag="wk")
        wqv_sb = cpool.tile([P, KC, 2, HD], BF16, tag="wqv")
        wo_sb = cpool.tile([P, D_MODEL], BF16, tag="wo")
        bias3_sb = cpool.tile([P, 3], FP32, tag="bias3")
        zb_sb = bias3_sb[:, 0:1]
        bk_sb = bias3_sb[:, 1:2]
        bq_sb = bias3_sb[:, 2:3]
        wq_sb = wqv_sb[:, :, 0]
        wv_sb = wqv_sb[:, :, 1]
        nc.sync.dma_start(wk_sb, io["wkp"].rearrange("p (kc m) -> p kc m", kc=KC))
        nc.sync.dma_start(bias3_sb, io["bias3"])

        # --- persistent activations ---
        qhT = [apool.tile([P, S], BF16, tag=f"qhT{b}", name=f"qhT{b}") for b in range(B)]
        khT = [apool.tile([P, S], BF16, tag=f"khT{b}", name=f"khT{b}") for b in range(B)]
        # vh: [part=seq%128, global k-tile, head, 64 depth + ones col]
        vh = apool.tile([P, B * KT, HPC, DEPTH + 1], BF16, tag="vh")
        nc.vector.memset(vh[:, :, :, DEPTH : DEPTH + 1], 1.0)
        att = [apool.tile([P, S], BF16, tag=f"att{b}", name=f"att{b}") for b in range(B)]

        xtiles = {}

        def dma_x(name, b, c):
            t = xpool.tile([P, KC, NT], BF16, tag="xt", name=f"x_{name}_{b}_{c}")
            g0 = b * S + c * NT
            nc.sync.dma_start(
                t, io[name][:, g0 : g0 + NT].rearrange("(kc p) n -> p kc n", p=P)
            )
            xtiles[(name, b, c)] = t

        def dma_x_h(name, b, c, half):
            # 256-token half chunk (startup latency path)
            HT = NT // 2
            t = xpool.tile(
                [P, KC, HT], BF16, tag="xt", name=f"xh_{name}_{b}_{c}_{half}",
                padded_shape=[P, KC, NT],
            )
            g0 = b * S + c * NT + half * HT
            nc.sync.dma_start(
                t, io[name][:, g0 : g0 + HT].rearrange("(kc p) n -> p kc n", p=P)
            )
            xtiles[(name, b, c, half)] = t

        def qkproj(name, w_sb, b_sb, dst_list, b, c, half):
            # 256-token half-chunks keep PE blobs small between scores groups
            HT = NT // 2
            if (name, b, c, half) in xtiles:
                xt = xtiles[(name, b, c, half)]
                rhs_of = lambda kc: xt[:, kc]
            else:
                xt = xtiles[(name, b, c)]
                hs = slice(half * HT, (half + 1) * HT)
                rhs_of = lambda kc: xt[:, kc, hs]
            psq = popool.tile([P, HT], FP32, tag="po", name="psq", padded_shape=[P, NT])
            for kc in range(KC):
                nc.tensor.matmul(
                    psq, lhsT=w_sb[:, kc], rhs=rhs_of(kc),
                    start=(kc == 0), stop=(kc == KC - 1),
                )
            g0 = c * NT + half * HT
            nc.vector.tensor_scalar_add(
                dst_list[b][:, g0 : g0 + HT], psq, b_sb
            )

        def vproj(b, t):  # one 128-token tile -> vh[:, b*KT+t]
            c, t4 = divmod(t, 4)
            xt = xtiles[("xvT", b, c)]
            psv = popool.tile([P, NT], FP32, tag="po", name="psv")
            for kc in range(KC):
                nc.tensor.matmul(
                    psv[:, 0:HD],
                    lhsT=xt[:, kc, t4 * P : (t4 + 1) * P],
                    rhs=wv_sb[:, kc],
                    start=(kc == 0), stop=(kc == KC - 1),
                )
            for h in range(HPC):
                nc.vector.tensor_copy(
                    vh[:, b * KT + t, h, 0:DEPTH],
                    psv[:, h * DEPTH : (h + 1) * DEPTH],
                )

        # --- attention pipeline pieces ---
        pavs = {}  # (b, qt, h) -> [P, 4, DEPTH+1] accumulator

        def scores_exp(b, qt, kt):
            # one kt per group; BOTH heads share one [P, 2, NT] psum tile so
            # a bufs=2 ring truly double-buffers scores against exp
            qc = slice(qt * NT, (qt + 1) * NT)
            pss = pspool.tile([P, 2, NT], FP32, tag="ps", name="pss")
            es = spool.tile([P, 2, NT], BF16, tag="es", name="es")
            for h in range(HPC):
                hs = slice(h * DEPTH, (h + 1) * DEPTH)
                nc.tensor.matmul(
                    pss[:, h],
                    lhsT=khT[b][hs, kt * P : (kt + 1) * P],
                    rhs=qhT[b][hs, qc],
                    start=True, stop=True,
                    skip_group_check=True,
                )
            nc.scalar.activation(es, pss, EXP, bias=zb_sb, scale=0.125)
            return es

        def av(b, qt, kt, es):
            for h in range(HPC):
                if (b, qt, h) not in pavs:
                    pavs[(b, qt, h)] = pavpool.tile(
                        [P, NT], FP32, tag="pav", name=f"pav{h}"
                    )
                nc.tensor.matmul(
                    pavs[(b, qt, h)][0 : DEPTH + 1, :],
                    lhsT=vh[:, b * KT + kt, h, 0 : DEPTH + 1],
                    rhs=es[:, h],
                    start=(kt == 0), stop=(kt == KT - 1),
                )

        def normalize(b, qt, tail=False):
            qc = slice(qt * NT, (qt + 1) * NT)
            if tail:
                # latency-optimized: per-128q chaining so the tail outproj of
                # qs4=0 starts before the full 512-wide normalize completes
                rcs = []
                for h in range(HPC):
                    rc = smpool.tile([1, NT], FP32, tag="rc")
                    nc.vector.reciprocal(rc, pavs[(b, qt, h)][DEPTH : DEPTH + 1, :])
                    rcs.append(rc)
                for qs4 in range(4):
                    qsl = slice(qs4 * P, (qs4 + 1) * P)
                    for h in range(HPC):
                        hs = slice(h * DEPTH, (h + 1) * DEPTH)
                        rb = smpool.tile([DEPTH, P], FP32, tag="rb4")
                        nc.gpsimd.partition_broadcast(rb, rcs[h][:, qsl])
                        nc.vector.tensor_mul(
                            att[b][hs, qt * NT + qs4 * P : qt * NT + (qs4 + 1) * P],
                            pavs[(b, qt, h)][0:DEPTH, qsl],
                            rb,
                        )
                for h in range(HPC):
                    del pavs[(b, qt, h)]
                return
            # copy pav out first: the copy is pav's last reader, releasing
            # its psum slot ~1.3us earlier than the recip/broadcast/mul chain
            # would (the next q-group's AV start=True waits on that slot)
            pcops = []
            for h in range(HPC):
                pcop = smpool.tile([P, NT], FP32, tag="pcop")
                nc.vector.tensor_copy(
                    pcop[0 : DEPTH + 1, :], pavs[(b, qt, h)][0 : DEPTH + 1, :]
                )
                pcops.append(pcop)
            for h in range(HPC):
                hs = slice(h * DEPTH, (h + 1) * DEPTH)
                rc = smpool.tile([1, NT], FP32, tag="rc")
                nc.vector.reciprocal(rc, pcops[h][DEPTH : DEPTH + 1, :])
                rb = smpool.tile([DEPTH, NT], FP32, tag="rb")
                nc.gpsimd.partition_broadcast(rb, rc)
                nc.vector.tensor_mul(att[b][hs, qc], pcops[h][0:DEPTH, :], rb)
            for h in range(HPC):
                del pavs[(b, qt, h)]

        def outproj(b, qt, tail=False):
            if tail:
                for qs4 in range(4):
                    lhs = att[b][:, qt * NT + qs4 * P : qt * NT + (qs4 + 1) * P]
                    ot4 = opool.tile([P, D_MODEL], BF16, tag="ot4", bufs=4)
                    psos = []
                    for mi in range(2):
                        pso = popool.tile([P, NT], FP32, tag="po", name="pso")
                        nc.tensor.matmul(
                            pso, lhsT=lhs,
                            rhs=wo_sb[:, mi * NT : (mi + 1) * NT],
                            start=True, stop=True,
                        )
                        psos.append(pso)
                    nc.vector.tensor_copy(ot4[:, 0:NT], psos[0])
                    nc.scalar.copy(ot4[:, NT : 2 * NT], psos[1])
                    r0 = b * S + qt * NT + qs4 * P
                    nc.sync.dma_start(io["out"][r0 : r0 + P, :], ot4)
                return
            ot = opool.tile([P, 4, D_MODEL], BF16, tag="ot")
            for qs4 in range(4):
                lhs = att[b][:, qt * NT + qs4 * P : qt * NT + (qs4 + 1) * P]
                for mi in range(2):
                    pso = popool.tile([P, NT], FP32, tag="po", name="pso")
                    nc.tensor.matmul(
                        pso, lhsT=lhs, rhs=wo_sb[:, mi * NT : (mi + 1) * NT],
                        start=True, stop=True,
                    )
                    nc.vector.tensor_copy(ot[:, qs4, mi * NT : (mi + 1) * NT], pso)
            r0 = b * S + qt * NT
            nc.sync.dma_start(
                io["out"][r0 : r0 + NT, :].rearrange("(qs p) m -> p qs m", p=P), ot
            )

        # --- schedule ---
        # Background-task placement: tasks[i] emitted just before stream[i].
        tasks = defaultdict(list)

        def at(i, fn, *a):
            tasks[i].append((fn, a))

        # batch-0 startup (before the stream): interleave weight/x DMAs so the
        # first scores group has its data ~7us in; wo + remaining chunks later.
        dma_x_h("xkT", 0, 0, 0)
        nc.sync.dma_start(
            wqv_sb, io["wqv"].rearrange("p (kc two m) -> p kc two m", kc=KC, two=2)
        )
        dma_x_h("xqT", 0, 0, 0)
        dma_x_h("xqT", 0, 0, 1)
        dma_x_h("xkT", 0, 0, 1)
        dma_x("xvT", 0, 0)
        qkproj("xkT", wk_sb, bk_sb, khT, 0, 0, 0)
        qkproj("xqT", wq_sb, bq_sb, qhT, 0, 0, 0)
        qkproj("xqT", wq_sb, bq_sb, qhT, 0, 0, 1)
        qkproj("xkT", wk_sb, bk_sb, khT, 0, 0, 1)

        # remaining x-DMAs, emitted early on the SP queue (no dependent DMAs
        # ahead of them -> no head-of-line blocking; xpool ring gives backpressure)
        at(0, dma_x, "xkT", 0, 1)
        at(1, dma_x, "xvT", 0, 1)
        at(2, dma_x, "xkT", 0, 2)
        at(3, dma_x, "xvT", 0, 2)
        at(4, dma_x, "xkT", 0, 3)
        at(5, dma_x, "xvT", 0, 3)
        at(6, dma_x, "xqT", 0, 1)
        at(7, nc.sync.dma_start, wo_sb, io["woT"])
        at(8, dma_x, "xqT", 0, 2)
        at(10, dma_x, "xqT", 0, 3)
        at(16, dma_x, "xkT", 1, 0)
        at(20, dma_x, "xkT", 1, 1)
        at(24, dma_x, "xkT", 1, 2)
        at(28, dma_x, "xkT", 1, 3)
        at(31, dma_x, "xqT", 1, 0)
        at(35, dma_x, "xvT", 1, 0)
        at(39, dma_x, "xvT", 1, 1)
        at(43, dma_x, "xvT", 1, 2)
        at(46, dma_x, "xvT", 1, 3)
        at(51, dma_x, "xqT", 1, 1)
        at(55, dma_x, "xqT", 1, 2)
        at(59, dma_x, "xqT", 1, 3)

        # b0 proj chunks, just-in-time (deadlines with post-scores task order:
        # kproj c by idx 4c-1, vproj t by idx t, qproj c by idx 16c-1)
        at(1, vproj, 0, 0)
        at(1, vproj, 0, 1)
        at(2, qkproj, "xkT", wk_sb, bk_sb, khT, 0, 1, 0)
        at(3, vproj, 0, 2)
        at(3, vproj, 0, 3)
        at(4, vproj, 0, 4)
        at(4, qkproj, "xkT", wk_sb, bk_sb, khT, 0, 1, 1)
        at(5, vproj, 0, 5)
        at(5, vproj, 0, 6)
        at(6, qkproj, "xkT", wk_sb, bk_sb, khT, 0, 2, 0)
        at(6, vproj, 0, 7)
        at(7, vproj, 0, 8)
        at(7, qkproj, "xkT", wk_sb, bk_sb, khT, 0, 2, 1)
        at(8, vproj, 0, 9)
        at(9, vproj, 0, 10)
        at(9, qkproj, "xkT", wk_sb, bk_sb, khT, 0, 3, 0)
        at(10, vproj, 0, 11)
        at(11, vproj, 0, 12)
        at(11, qkproj, "xkT", wk_sb, bk_sb, khT, 0, 3, 1)
        at(12, qkproj, "xqT", wq_sb, bq_sb, qhT, 0, 1, 0)
        at(12, vproj, 0, 13)
        at(13, vproj, 0, 14)
        at(13, qkproj, "xqT", wq_sb, bq_sb, qhT, 0, 1, 1)
        at(14, vproj, 0, 15)
        at(18, qkproj, "xqT", wq_sb, bq_sb, qhT, 0, 2, 0)
        at(20, qkproj, "xqT", wq_sb, bq_sb, qhT, 0, 2, 1)
        at(33, qkproj, "xqT", wq_sb, bq_sb, qhT, 0, 3, 0)
        at(35, qkproj, "xqT", wq_sb, bq_sb, qhT, 0, 3, 1)

        # batch-1 projections, spread through b0's qt2..qt3
        at(32, qkproj, "xkT", wk_sb, bk_sb, khT, 1, 0, 0)
        at(34, qkproj, "xkT", wk_sb, bk_sb, khT, 1, 0, 1)
        at(36, qkproj, "xkT", wk_sb, bk_sb, khT, 1, 1, 0)
        at(38, qkproj, "xkT", wk_sb, bk_sb, khT, 1, 1, 1)
        at(40, qkproj, "xkT", wk_sb, bk_sb, khT, 1, 2, 0)
        at(42, qkproj, "xkT", wk_sb, bk_sb, khT, 1, 2, 1)
        at(44, qkproj, "xkT", wk_sb, bk_sb, khT, 1, 3, 0)
        at(45, qkproj, "xkT", wk_sb, bk_sb, khT, 1, 3, 1)
        at(46, qkproj, "xqT", wq_sb, bq_sb, qhT, 1, 0, 0)
        at(47, qkproj, "xqT", wq_sb, bq_sb, qhT, 1, 0, 1)
        for t in range(16):
            at(48 + t, vproj, 1, t)
        at(65, qkproj, "xqT", wq_sb, bq_sb, qhT, 1, 1, 0)
        at(67, qkproj, "xqT", wq_sb, bq_sb, qhT, 1, 1, 1)
        at(81, qkproj, "xqT", wq_sb, bq_sb, qhT, 1, 2, 0)
        at(83, qkproj, "xqT", wq_sb, bq_sb, qhT, 1, 2, 1)
        at(97, qkproj, "xqT", wq_sb, bq_sb, qhT, 1, 3, 0)
        at(99, qkproj, "xqT", wq_sb, bq_sb, qhT, 1, 3, 1)

        stream = [
            (b, qt, kt) for b in range(B) for qt in range(QT) for kt in range(KT)
        ]
        # 1-stage software pipeline: scores/exp(i) || AV(i-1); finalize of a
        # q-group is split so the out-proj trails its DMA transpose.
        # 2-deep software pipeline: av(i) is emitted two groups after its
        # scores/exp so the in-order PE always has >=2 groups of independent
        # work queued ahead of any exp-gated ldweights.
        PIPE = 4
        OP_DELAY = 4  # flushes between normalize (XBAR issue) and outproj
        hist = []  # [(b, qt, kt, es), ...]
        pending_fins = []  # [[countdown, (b, qt, atts)], ...]

        def flush_one():
            pb, pqt, pkt, pes = hist.pop(0)
            last = (pb, pqt) == (B - 1, QT - 1)
            av(pb, pqt, pkt, pes)
            for pf in pending_fins:
                pf[0] -= 1
            while pending_fins and pending_fins[0][0] <= 0:
                outproj(*pending_fins.pop(0)[1])
            if pkt == KT - 1:
                normalize(pb, pqt, tail=last)
                if last:
                    outproj(pb, pqt, tail=True)
                else:
                    pending_fins.append([OP_DELAY, (pb, pqt)])

        for i, (b, qt, kt) in enumerate(stream):
            es = scores_exp(b, qt, kt)
            hist.append((b, qt, kt, es))
            if len(hist) > PIPE:
                flush_one()
            # background proj/DMA tasks go after the critical scores->exp chain
            for fn, a in tasks.get(i, ()):
                fn(*a)
        while hist:
            flush_one()
        while pending_fins:
            outproj(*pending_fins.pop(0)[1])


_NC_CACHE = None


def get_nc():
    global _NC_CACHE
    if _NC_CACHE is None:
        _NC_CACHE = _build_program()
    return _NC_CACHE


def make_in_maps(q, k, v, wq, bq, wk, bk, wv, bv, wo, bo):
    """Host-side shard/transpose/cast prep. Returns (in_maps, host_bias_row)."""
    f32 = np.float32
    qT = np.ascontiguousarray(np.asarray(q, f32).reshape(BS, D_MODEL).T).astype(NPBF)
    kT = np.ascontiguousarray(np.asarray(k, f32).reshape(BS, D_MODEL).T).astype(NPBF)
    vT = np.ascontiguousarray(np.asarray(v, f32).reshape(BS, D_MODEL).T).astype(NPBF)
    wq, wk, wv, wo = (np.asarray(a, f32) for a in (wq, wk, wv, wo))
    bq, bk, bv, bo = (np.asarray(a, f32) for a in (bq, bk, bv, bo))
    def pack_w(wT):
        # [D_MODEL, HD] -> [P, KC, HD]: row p holds [kc, hd] for dm = kc*128+p
        return wT.reshape(KC, P, HD).transpose(1, 0, 2)

    in_maps = []
    for c in range(NCORES):
        sl = slice(c * HD, (c + 1) * HD)
        wqv_p = np.stack(
            [pack_w(wq[sl, :].T), pack_w(wv[sl, :].T)], axis=2
        )  # [P, KC, 2, HD]
        bias3 = np.stack(
            [np.zeros(HD, np.float32), bk[sl], bq[sl]], axis=1
        ).astype(np.float32)
        in_maps.append(
            {
                "xqT": qT,
                "xkT": kT,
                "xvT": vT,
                "wkp": np.ascontiguousarray(
                    pack_w(wk[sl, :].T).reshape(P, KC * HD)
                ).astype(NPBF),
                "wqv": np.ascontiguousarray(wqv_p.reshape(P, KC * 2 * HD)).astype(NPBF),
                "woT": np.ascontiguousarray(wo[:, sl].T).astype(NPBF),
                "bias3": np.ascontiguousarray(bias3),
            }
        )
    # bv enters linearly (softmax rows sum to 1): out += bv @ wo.T + bo
    host_bias = (bv @ wo.T + bo).astype(f32)
    return in_maps, host_bias


def run_on_hw(inputs, trace=False, **kw):
    nc = get_nc()
    in_maps, host_bias = make_in_maps(**inputs)
    res = run_bass_kernel_spmd(
        nc, in_maps, list(range(NCORES)), trace=trace, **kw
    )
    acc = np.zeros((BS, D_MODEL), np.float32)
    for c in range(NCORES):
        acc += np.asarray(res.results[c]["out"], np.float32)
    acc += host_bias[None, :]
    return acc.reshape(B, S, D_MODEL), res


def kernel(**inputs):
    out, _ = run_on_hw(inputs, trace=False)
    return out


# revision 56
# speedup vs baseline: 1.0008x; 1.0008x over previous
"""Multi-head attention, tensor-parallel over heads on 8 Trainium2 NeuronCores.

Contract: kernel(**inputs) takes the FULL unsharded inputs from
reference.setup_inputs() and returns the FULL [2, 2048, 1024] fp32 output.

Sharding: 16 heads / 8 cores = 2 heads per core (tensor parallel).
Each core receives the full (host-transposed, bf16-cast) activations and its
2-head slice of wq/wk/wv plus the matching wo columns; it computes
  qhT/khT = (x @ Wq_c^T)^T      (head-dim on partitions)
  vh      =  x @ Wv_c^T          (seq on partitions, direct orientation)
  S^T     = khT^T.T @ qhT        (k-seq on PSUM partitions, 1 kt per group,
                                  both heads sharing one psum tile so bufs=2
                                  truly double-buffers scores against exp)
  es      = exp(S^T / 8)         (no max subtraction: logits ~ N(0,1))
  att^T   = [vh | 1]^T-stationary AV matmul accumulated over kt (row 64 =
             softmax denominator; vh must be the STATIONARY operand: the
             lowering's Ldweights split does not carry semaphore waits, so a
             freshly-written stationary like es races on the in-order PE)
  att     = att^T[:64] * gpsimd-broadcast(recip(att^T[64]))
  partial = att^T @ wo_c^T       (stored bf16)
The host sums the 8 partials and adds the (linear) bv/bo bias terms.

Emission is software-pipelined (PIPE=4): scores/exp of group i are emitted
four groups before their AV matmuls so the in-order PE queue always has
independent work ahead of any exp-gated instruction; the out-projection of a
q-group trails its normalize by OP_DELAY flushes. Background projection work
and x prefetch for the next batch are interleaved at group boundaries on the
SP DMA queue (dependent DMAs never head-of-line block the prefetch), and the
exp zero-bias/weights ride in packed input tensors so no const-AP DMA stalls
the first exp.
"""

import sys
from collections import defaultdict

import numpy as np

sys.path.insert(0, "/opt/trn_rl_repo")

import ml_dtypes  # noqa: E402

import concourse.bacc as bacc  # noqa: E402
import concourse.mybir as mybir  # noqa: E402
import concourse.tile as tile  # noqa: E402
from concourse.bass_utils import run_bass_kernel_spmd  # noqa: E402

D_MODEL = 1024
NUM_HEADS = 16
DEPTH = 64
B, S = 2, 2048
BS = B * S  # 4096
NCORES = 8
HPC = NUM_HEADS // NCORES  # 2 heads per core
HD = HPC * DEPTH  # 128 head dims per core
KC = D_MODEL // 128  # 8 contraction chunks of 128
NT = 512  # x-chunk / moving-free tile
QT = S // NT  # 4 q-groups per batch
KT = S // 128  # 16 k-tiles per batch
KT2 = KT // 2  # 8 kt-pair groups per q-group
P = 128

FP32 = mybir.dt.float32
BF16 = mybir.dt.bfloat16
NPBF = ml_dtypes.bfloat16


def _build_program(loop_iters=1):
    nc = bacc.Bacc(
        "TRN2", target_bir_lowering=False, debug=False, num_devices=NCORES
    )
    io = {}
    io["xqT"] = nc.dram_tensor("xqT", [D_MODEL, BS], BF16, kind="ExternalInput").ap()
    io["xkT"] = nc.dram_tensor("xkT", [D_MODEL, BS], BF16, kind="ExternalInput").ap()
    io["xvT"] = nc.dram_tensor("xvT", [D_MODEL, BS], BF16, kind="ExternalInput").ap()
    # qkv weights packed host-side as [p, kc, 3, hd] (one 6KB-row DMA);
    # bias vectors packed as [p, 3] = (zb, bk, bq)
    io["wkp"] = nc.dram_tensor("wkp", [P, KC * HD], BF16, kind="ExternalInput").ap()
    io["wqv"] = nc.dram_tensor(
        "wqv", [P, KC * 2 * HD], BF16, kind="ExternalInput"
    ).ap()
    io["woT"] = nc.dram_tensor("woT", [HD, D_MODEL], BF16, kind="ExternalInput").ap()
    io["bias3"] = nc.dram_tensor("bias3", [HD, 3], FP32, kind="ExternalInput").ap()
    io["out"] = nc.dram_tensor("out", [BS, D_MODEL], BF16, kind="ExternalOutput").ap()

    with tile.TileContext(nc, trace_sim=False) as tc:
        if loop_iters > 1:
            with tc.For_i(0, loop_iters, 1):
                _emit(tc, nc, io)
        else:
            _emit(tc, nc, io)
    nc.compile()
    return nc


def _emit(tc, nc, io):
    EXP = mybir.ActivationFunctionType.Exp
    with (
        tc.tile_pool(name="const", bufs=1) as cpool,
        tc.tile_pool(name="acts", bufs=1) as apool,
        tc.tile_pool(name="xin", bufs=10) as xpool,
        tc.tile_pool(name="es", bufs=7) as spool,
        tc.tile_pool(name="sm", bufs=4) as smpool,
        tc.tile_pool(name="ot", bufs=2) as opool,
        tc.tile_pool(name="ps", bufs=2, space="PSUM") as pspool,
        tc.tile_pool(name="pv", bufs=2, space="PSUM") as pavpool,
        tc.tile_pool(name="po", bufs=2, space="PSUM") as popool,
    ):
        # --- constants (emission order = DMA service order on the shared bus;
        #     bias+qkv weights + first x chunks first, wo late). The zero exp
        #     bias rides in bias3 col 0: a float bias would lower to a
        #     const-AP DMA queued behind all startup x DMAs.
        wk_sb = cpool.tile([P, KC, HD], BF16, tag="wk")
        wqv_sb = cpool.tile([P, KC, 2, HD], BF16, tag="wqv")
        wo_sb = cpool.tile([P, D_MODEL], BF16, tag="wo")
        bias3_sb = cpool.tile([P, 3], FP32, tag="bias3")
        zb_sb = bias3_sb[:, 0:1]
        bk_sb = bias3_sb[:, 1:2]
        bq_sb = bias3_sb[:, 2:3]
        wq_sb = wqv_sb[:, :, 0]
        wv_sb = wqv_sb[:, :, 1]
        nc.sync.dma_start(wk_sb, io["wkp"].rearrange("p (kc m) -> p kc m", kc=KC))
        nc.sync.dma_start(bias3_sb, io["bias3"])

        # --- persistent activations ---
        qhT = [apool.tile([P, S], BF16, tag=f"qhT{b}", name=f"qhT{b}") for b in range(B)]
        khT = [apool.tile([P, S], BF16, tag=f"khT{b}", name=f"khT{b}") for b in range(B)]
        # vh: [part=seq%128, global k-tile, head, 64 depth + ones col]
        vh = apool.tile([P, B * KT, HPC, DEPTH + 1], BF16, tag="vh")
        nc.vector.memset(vh[:, :, :, DEPTH : DEPTH + 1], 1.0)
        att = [apool.tile([P, S], BF16, tag=f"att{b}", name=f"att{b}") for b in range(B)]

        xtiles = {}

        def dma_x(name, b, c):
            t = xpool.tile([P, KC, NT], BF16, tag="xt", name=f"x_{name}_{b}_{c}")
            g0 = b * S + c * NT
            nc.sync.dma_start(
                t, io[name][:, g0 : g0 + NT].rearrange("(kc p) n -> p kc n", p=P)
            )
            xtiles[(name, b, c)] = t

        def dma_x_h(name, b, c, half):
            # 256-token half chunk (startup latency path)
            HT = NT // 2
            t = xpool.tile(
                [P, KC, HT], BF16, tag="xt", name=f"xh_{name}_{b}_{c}_{half}",
                padded_shape=[P, KC, NT],
            )
            g0 = b * S + c * NT + half * HT
            nc.sync.dma_start(
                t, io[name][:, g0 : g0 + HT].rearrange("(kc p) n -> p kc n", p=P)
            )
            xtiles[(name, b, c, half)] = t

        def qkproj(name, w_sb, b_sb, dst_list, b, c, half):
            # 256-token half-chunks keep PE blobs small between scores groups
            HT = NT // 2
            if (name, b, c, half) in xtiles:
                xt = xtiles[(name, b, c, half)]
                rhs_of = lambda kc: xt[:, kc]
            else:
                xt = xtiles[(name, b, c)]
                hs = slice(half * HT, (half + 1) * HT)
                rhs_of = lambda kc: xt[:, kc, hs]
            psq = popool.tile([P, HT], FP32, tag="po", name="psq", padded_shape=[P, NT])
            for kc in range(KC):
                nc.tensor.matmul(
                    psq, lhsT=w_sb[:, kc], rhs=rhs_of(kc),
                    start=(kc == 0), stop=(kc == KC - 1),
                )
            g0 = c * NT + half * HT
            nc.vector.tensor_scalar_add(
                dst_list[b][:, g0 : g0 + HT], psq, b_sb
            )

        def vproj(b, t):  # one 128-token tile -> vh[:, b*KT+t]
            c, t4 = divmod(t, 4)
            xt = xtiles[("xvT", b, c)]
            psv = popool.tile([P, NT], FP32, tag="po", name="psv")
            for kc in range(KC):
                nc.tensor.matmul(
                    psv[:, 0:HD],
                    lhsT=xt[:, kc, t4 * P : (t4 + 1) * P],
                    rhs=wv_sb[:, kc],
                    start=(kc == 0), stop=(kc == KC - 1),
                )
            for h in range(HPC):
                nc.vector.tensor_copy(
                    vh[:, b * KT + t, h, 0:DEPTH],
                    psv[:, h * DEPTH : (h + 1) * DEPTH],
                )

        # --- attention pipeline pieces ---
        pavs = {}  # (b, qt, h) -> [P, 4, DEPTH+1] accumulator

        def scores_exp(b, qt, kt):
            # one kt per group; BOTH heads share one [P, 2, NT] psum tile so
            # a bufs=2 ring truly double-buffers scores against exp
            qc = slice(qt * NT, (qt + 1) * NT)
            pss = pspool.tile([P, 2, NT], FP32, tag="ps", name="pss")
            es = spool.tile([P, 2, NT], BF16, tag="es", name="es")
            for h in range(HPC):
                hs = slice(h * DEPTH, (h + 1) * DEPTH)
                nc.tensor.matmul(
                    pss[:, h],
                    lhsT=khT[b][hs, kt * P : (kt + 1) * P],
                    rhs=qhT[b][hs, qc],
                    start=True, stop=True,
                    skip_group_check=True,
                )
            nc.scalar.activation(es, pss, EXP, bias=zb_sb, scale=0.125)
            return es

        def av(b, qt, kt, es):
            for h in range(HPC):
                if (b, qt, h) not in pavs:
                    pavs[(b, qt, h)] = pavpool.tile(
                        [P, NT], FP32, tag="pav", name=f"pav{h}"
                    )
                nc.tensor.matmul(
                    pavs[(b, qt, h)][0 : DEPTH + 1, :],
                    lhsT=vh[:, b * KT + kt, h, 0 : DEPTH + 1],
                    rhs=es[:, h],
                    start=(kt == 0), stop=(kt == KT - 1),
                )

        def normalize(b, qt, tail=False):
            qc = slice(qt * NT, (qt + 1) * NT)
            if tail:
                # latency-optimized: per-128q chaining so the tail outproj of
                # qs4=0 starts before the full 512-wide normalize completes
                rcs = []
                for h in range(HPC):
                    rc = smpool.tile([1, NT], FP32, tag="rc")
                    nc.vector.reciprocal(rc, pavs[(b, qt, h)][DEPTH : DEPTH + 1, :])
                    rcs.append(rc)
                for qs4 in range(4):
                    qsl = slice(qs4 * P, (qs4 + 1) * P)
                    for h in range(HPC):
                        hs = slice(h * DEPTH, (h + 1) * DEPTH)
                        rb = smpool.tile([DEPTH, P], FP32, tag="rb4")
                        nc.gpsimd.partition_broadcast(rb, rcs[h][:, qsl])
                        nc.vector.tensor_mul(
                            att[b][hs, qt * NT + qs4 * P : qt * NT + (qs4 + 1) * P],
                            pavs[(b, qt, h)][0:DEPTH, qsl],
                            rb,
                        )
                for h in range(HPC):
                    del pavs[(b, qt, h)]
                return
            # copy pav out first: the copy is pav's last reader, releasing
            # its psum slot ~1.3us earlier than the recip/broadcast/mul chain
            # would (the next q-group's AV start=True waits on that slot)
            pcops = []
            for h in range(HPC):
                pcop = smpool.tile([P, NT], FP32, tag="pcop")
                nc.vector.tensor_copy(
                    pcop[0 : DEPTH + 1, :], pavs[(b, qt, h)][0 : DEPTH + 1, :]
                )
                pcops.append(pcop)
            for h in range(HPC):
                hs = slice(h * DEPTH, (h + 1) * DEPTH)
                rc = smpool.tile([1, NT], FP32, tag="rc")
                nc.vector.reciprocal(rc, pcops[h][DEPTH : DEPTH + 1, :])
                rb = smpool.tile([DEPTH, NT], FP32, tag="rb")
                nc.gpsimd.partition_broadcast(rb, rc)
                nc.vector.tensor_mul(att[b][hs, qc], pcops[h][0:DEPTH, :], rb)
            for h in range(HPC):
                del pavs[(b, qt, h)]

        def outproj(b, qt, tail=False):
            if tail:
                for qs4 in range(4):
                    lhs = att[b][:, qt * NT + qs4 * P : qt * NT + (qs4 + 1) * P]
                    ot4 = opool.tile([P, D_MODEL], BF16, tag="ot4", bufs=4)
                    psos = []
                    for mi in range(2):
                        pso = popool.tile([P, NT], FP32, tag="po", name="pso")
                        nc.tensor.matmul(
                            pso, lhsT=lhs,
                            rhs=wo_sb[:, mi * NT : (mi + 1) * NT],
                            start=True, stop=True,
                        )
                        psos.append(pso)
                    nc.vector.tensor_copy(ot4[:, 0:NT], psos[0])
                    nc.scalar.copy(ot4[:, NT : 2 * NT], psos[1])
                    r0 = b * S + qt * NT + qs4 * P
                    nc.sync.dma_start(io["out"][r0 : r0 + P, :], ot4)
                return
            ot = opool.tile([P, 4, D_MODEL], BF16, tag="ot")
            for qs4 in range(4):
                lhs = att[b][:, qt * NT + qs4 * P : qt * NT + (qs4 + 1) * P]
                for mi in range(2):
                    pso = popool.tile([P, NT], FP32, tag="po", name="pso")
                    nc.tensor.matmul(
                        pso, lhsT=lhs, rhs=wo_sb[:, mi * NT : (mi + 1) * NT],
                        start=True, stop=True,
                    )
                    nc.vector.tensor_copy(ot[:, qs4, mi * NT : (mi + 1) * NT], pso)
            r0 = b * S + qt * NT
            nc.sync.dma_start(
                io["out"][r0 : r0 + NT, :].rearrange("(qs p) m -> p qs m", p=P), ot
            )

        # --- schedule ---
        # Background-task placement: tasks[i] emitted just before stream[i].
        tasks = defaultdict(list)

        def at(i, fn, *a):
            tasks[i].append((fn, a))

        # batch-0 startup (before the stream): interleave weight/x DMAs so the
        # first scores group has its data ~7us in; wo + remaining chunks later.
        dma_x_h("xkT", 0, 0, 0)
        nc.sync.dma_start(
            wqv_sb, io["wqv"].rearrange("p (kc two m) -> p kc two m", kc=KC, two=2)
        )
        dma_x("xqT", 0, 0)
        dma_x_h("xkT", 0, 0, 1)
        dma_x("xvT", 0, 0)
        qkproj("xkT", wk_sb, bk_sb, khT, 0, 0, 0)
        qkproj("xqT", wq_sb, bq_sb, qhT, 0, 0, 0)
        qkproj("xqT", wq_sb, bq_sb, qhT, 0, 0, 1)
        qkproj("xkT", wk_sb, bk_sb, khT, 0, 0, 1)

        # remaining x-DMAs, emitted early on the SP queue (no dependent DMAs
        # ahead of them -> no head-of-line blocking; xpool ring gives backpressure)
        at(0, dma_x, "xkT", 0, 1)
        at(1, dma_x, "xvT", 0, 1)
        at(2, dma_x, "xkT", 0, 2)
        at(3, dma_x, "xvT", 0, 2)
        at(4, dma_x, "xkT", 0, 3)
        at(5, dma_x, "xvT", 0, 3)
        at(6, dma_x, "xqT", 0, 1)
        at(7, nc.sync.dma_start, wo_sb, io["woT"])
        at(8, dma_x, "xqT", 0, 2)
        at(10, dma_x, "xqT", 0, 3)
        at(16, dma_x, "xkT", 1, 0)
        at(20, dma_x, "xkT", 1, 1)
        at(24, dma_x, "xkT", 1, 2)
        at(28, dma_x, "xkT", 1, 3)
        at(31, dma_x, "xqT", 1, 0)
        at(35, dma_x, "xvT", 1, 0)
        at(39, dma_x, "xvT", 1, 1)
        at(43, dma_x, "xvT", 1, 2)
        at(46, dma_x, "xvT", 1, 3)
        at(51, dma_x, "xqT", 1, 1)
        at(55, dma_x, "xqT", 1, 2)
        at(59, dma_x, "xqT", 1, 3)

        # b0 proj chunks, just-in-time (deadlines with post-scores task order:
        # kproj c by idx 4c-1, vproj t by idx t, qproj c by idx 16c-1)
        at(1, vproj, 0, 0)
        at(1, vproj, 0, 1)
        at(2, qkproj, "xkT", wk_sb, bk_sb, khT, 0, 1, 0)
        at(3, vproj, 0, 2)
        at(3, vproj, 0, 3)
        at(4, vproj, 0, 4)
        at(4, qkproj, "xkT", wk_sb, bk_sb, khT, 0, 1, 1)
        at(5, vproj, 0, 5)
        at(5, vproj, 0, 6)
        at(6, qkproj, "xkT", wk_sb, bk_sb, khT, 0, 2, 0)
        at(6, vproj, 0, 7)
        at(7, vproj, 0, 8)
        at(7, qkproj, "xkT", wk_sb, bk_sb, khT, 0, 2, 1)
        at(8, vproj, 0, 9)
        at(9, vproj, 0, 10)
        at(9, qkproj, "xkT", wk_sb, bk_sb, khT, 0, 3, 0)
        at(10, vproj, 0, 11)
        at(11, vproj, 0, 12)
        at(11, qkproj, "xkT", wk_sb, bk_sb, khT, 0, 3, 1)
        at(12, qkproj, "xqT", wq_sb, bq_sb, qhT, 0, 1, 0)
        at(12, vproj, 0, 13)
        at(13, vproj, 0, 14)
        at(13, qkproj, "xqT", wq_sb, bq_sb, qhT, 0, 1, 1)
        at(14, vproj, 0, 15)
        at(18, qkproj, "xqT", wq_sb, bq_sb, qhT, 0, 2, 0)
        at(20, qkproj, "xqT", wq_sb, bq_sb, qhT, 0, 2, 1)
        at(33, qkproj, "xqT", wq_sb, bq_sb, qhT, 0, 3, 0)
        at(35, qkproj, "xqT", wq_sb, bq_sb, qhT, 0, 3, 1)

        # batch-1 projections, spread through b0's qt2..qt3
        at(32, qkproj, "xkT", wk_sb, bk_sb, khT, 1, 0, 0)
        at(34, qkproj, "xkT", wk_sb, bk_sb, khT, 1, 0, 1)
        at(36, qkproj, "xkT", wk_sb, bk_sb, khT, 1, 1, 0)
        at(38, qkproj, "xkT", wk_sb, bk_sb, khT, 1, 1, 1)
        at(40, qkproj, "xkT", wk_sb, bk_sb, khT, 1, 2, 0)
        at(42, qkproj, "xkT", wk_sb, bk_sb, khT, 1, 2, 1)
        at(44, qkproj, "xkT", wk_sb, bk_sb, khT, 1, 3, 0)
        at(45, qkproj, "xkT", wk_sb, bk_sb, khT, 1, 3, 1)
        at(46, qkproj, "xqT", wq_sb, bq_sb, qhT, 1, 0, 0)
        at(47, qkproj, "xqT", wq_sb, bq_sb, qhT, 1, 0, 1)
        for t in range(16):
            at(48 + t, vproj, 1, t)
        at(65, qkproj, "xqT", wq_sb, bq_sb, qhT, 1, 1, 0)
        at(67, qkproj, "xqT", wq_sb, bq_sb, qhT, 1, 1, 1)
        at(81, qkproj, "xqT", wq_sb, bq_sb, qhT, 1, 2, 0)
        at(83, qkproj, "xqT", wq_sb, bq_sb, qhT, 1, 2, 1)
        at(97, qkproj, "xqT", wq_sb, bq_sb, qhT, 1, 3, 0)
        at(99, qkproj, "xqT", wq_sb, bq_sb, qhT, 1, 3, 1)

        stream = [
            (b, qt, kt) for b in range(B) for qt in range(QT) for kt in range(KT)
        ]
        # 1-stage software pipeline: scores/exp(i) || AV(i-1); finalize of a
        # q-group is split so the out-proj trails its DMA transpose.
        # 2-deep software pipeline: av(i) is emitted two groups after its
        # scores/exp so the in-order PE always has >=2 groups of independent
        # work queued ahead of any exp-gated ldweights.
        PIPE = 4
        OP_DELAY = 4  # flushes between normalize (XBAR issue) and outproj
        hist = []  # [(b, qt, kt, es), ...]
        pending_fins = []  # [[countdown, (b, qt, atts)], ...]

        def flush_one():
            pb, pqt, pkt, pes = hist.pop(0)
            last = (pb, pqt) == (B - 1, QT - 1)
            av(pb, pqt, pkt, pes)
            for pf in pending_fins:
                pf[0] -= 1
            while pending_fins and pending_fins[0][0] <= 0:
                outproj(*pending_fins.pop(0)[1])
            if pkt == KT - 1:
                normalize(pb, pqt, tail=last)
                if last:
                    outproj(pb, pqt, tail=True)
                else:
                    pending_fins.append([OP_DELAY, (pb, pqt)])

        for i, (b, qt, kt) in enumerate(stream):
            es = scores_exp(b, qt, kt)
            hist.append((b, qt, kt, es))
            if len(hist) > PIPE:
                flush_one()
            # background proj/DMA tasks go after the critical scores->exp chain
            for fn, a in tasks.get(i, ()):
                fn(*a)
        while hist:
            flush_one()
        while pending_fins:
            outproj(*pending_fins.pop(0)[1])


_NC_CACHE = None


def get_nc():
    global _NC_CACHE
    if _NC_CACHE is None:
        _NC_CACHE = _build_program()
    return _NC_CACHE


def make_in_maps(q, k, v, wq, bq, wk, bk, wv, bv, wo, bo):
    """Host-side shard/transpose/cast prep. Returns (in_maps, host_bias_row)."""
    f32 = np.float32
    qT = np.ascontiguousarray(np.asarray(q, f32).reshape(BS, D_MODEL).T).astype(NPBF)
    kT = np.ascontiguousarray(np.asarray(k, f32).reshape(BS, D_MODEL).T).astype(NPBF)
    vT = np.ascontiguousarray(np.asarray(v, f32).reshape(BS, D_MODEL).T).astype(NPBF)
    wq, wk, wv, wo = (np.asarray(a, f32) for a in (wq, wk, wv, wo))
    bq, bk, bv, bo = (np.asarray(a, f32) for a in (bq, bk, bv, bo))
    def pack_w(wT):
        # [D_MODEL, HD] -> [P, KC, HD]: row p holds [kc, hd] for dm = kc*128+p
        return wT.reshape(KC, P, HD).transpose(1, 0, 2)

    in_maps = []
    for c in range(NCORES):
        sl = slice(c * HD, (c + 1) * HD)
        wqv_p = np.stack(
            [pack_w(wq[sl, :].T), pack_w(wv[sl, :].T)], axis=2
        )  # [P, KC, 2, HD]
        bias3 = np.stack(
            [np.zeros(HD, np.float32), bk[sl], bq[sl]], axis=1
        ).astype(np.float32)
        in_maps.append(
            {
                "xqT": qT,
                "xkT": kT,
                "xvT": vT,
                "wkp": np.ascontiguousarray(
                    pack_w(wk[sl, :].T).reshape(P, KC * HD)
                ).astype(NPBF),
                "wqv": np.ascontiguousarray(wqv_p.reshape(P, KC * 2 * HD)).astype(NPBF),
                "woT": np.ascontiguousarray(wo[:, sl].T).astype(NPBF),
                "bias3": np.ascontiguousarray(bias3),
            }
        )
    # bv enters linearly (softmax rows sum to 1): out += bv @ wo.T + bo
    host_bias = (bv @ wo.T + bo).astype(f32)
    return in_maps, host_bias


def run_on_hw(inputs, trace=False, **kw):
    nc = get_nc()
    in_maps, host_bias = make_in_maps(**inputs)
    res = run_bass_kernel_spmd(
        nc, in_maps, list(range(NCORES)), trace=trace, **kw
    )
    acc = np.zeros((BS, D_MODEL), np.float32)
    for c in range(NCORES):
        acc += np.asarray(res.results[c]["out"], np.float32)
    acc += host_bias[None, :]
    return acc.reshape(B, S, D_MODEL), res


def kernel(**inputs):
    out, _ = run_on_hw(inputs, trace=False)
    return out
